# revision 1
# baseline (speedup 1.0000x reference)
"""GCN (2-layer, PyG-style gcn_norm) Bass/Tile kernel for Trainium2, 8 NeuronCores.

Strategy (dst-partitioned message passing):
  - Nodes are partitioned across 8 cores by destination; every edge is routed to
    the core that owns its destination node. Self-loops + symmetric D^-1/2 norm
    are computed on the host (index/routing preprocessing only).
  - Per core, edges are bucketed by source range (dma_gather indices are int16,
    so the feature table is addressed in <=32k-row buckets) and grouped by
    destination tile (128 dst nodes), packed into 128-message blocks.
  - Message features are bulk row-gathered from a replicated node-feature table
    in HBM with InstDMAGatherAnt (one call covers up to `gcols` blocks).
  - The segmented scatter-add becomes TensorE matmuls: for each block, a one-hot
    "selection" matrix S[e, d] = norm_e * (dst_local_e == d) is built with one
    DVE tensor_scalar op, and PSUM accumulates sum_e msg[e,:]^T S[e,:] over the
    tile's blocks. Bucket passes accumulate into a per-tile SBUF f32 buffer.
  - The layer weight matmul, bias+ReLU (ScalarE) and a TensorE transpose produce
    row-major output tiles, DMA'd to HBM.
  - Between layers, an AllGather shares the per-core H shards (the "halo
    exchange"); layer 2 gathers from the replicated table the same way.
"""

import os
from dataclasses import dataclass

import numpy as np

P = 128
NBUCK = 4  # source-range buckets (int16 gather indices => <=32768 rows each)


@dataclass(frozen=True)
class Geom:
    n_nodes: int
    n_cores: int
    in_dim: int
    h1: int
    h2: int
    gcols: int  # gather-group size, in 128-row blocks per dma_gather call
    mm_bf16: bool  # bf16 tables/matmul operands (accumulation stays f32)

    @property
    def shard(self) -> int:
        return -(-self.n_nodes // self.n_cores)

    @property
    def tiles(self) -> int:
        return -(-self.shard // P)

    @property
    def shard_pad(self) -> int:
        return self.tiles * P

    @property
    def bsz1(self) -> int:  # layer-1 table bucket size (x: n_nodes rows)
        return -(-self.n_nodes // NBUCK)

    @property
    def bsz2(self) -> int:  # layer-2 table bucket size (h_full rows)
        return -(-(self.n_cores * self.shard_pad) // NBUCK)


def preprocess(edge_index: np.ndarray, g: Geom):
    """Route edges to dst-owning cores; bucket by src range; pack into blocks.

    Returns (per_core, layout) where per_core[i] has gidx1/gidx2 (int16,
    [P, NB*8], dma_gather 16-wrapped), dl/v (f32 [P, NB]); layout has
    bpt (list over (bucket,tile) in stream order), calls [(c0, k, bucket)].
    """
    n, c, shard, tiles = g.n_nodes, g.n_cores, g.shard, g.tiles
    assert g.bsz1 <= 32768 and g.bsz2 <= 32768
    loops = np.arange(n, dtype=np.int64)
    src = np.concatenate([edge_index[0].astype(np.int64), loops])
    dst = np.concatenate([edge_index[1].astype(np.int64), loops])

    deg = np.bincount(dst, minlength=n).astype(np.float32)
    dinv = (1.0 / np.sqrt(deg)).astype(np.float32)  # deg >= 1 (self loops)
    norm = dinv[src] * dinv[dst]

    core = dst // shard
    local = dst - core * shard
    t_idx = local // P
    dl = (local % P).astype(np.float32)
    buck = src // g.bsz1
    src2 = (src // shard) * g.shard_pad + (src % shard)

    # stream order per core: bucket-major, then tile
    gkey = (core * NBUCK + buck) * tiles + t_idx
    ngrp = c * NBUCK * tiles
    cnt = np.bincount(gkey, minlength=ngrp).reshape(c, NBUCK, tiles)
    # uniform per-core program: blocks per (bucket, tile) = max over cores, >=1
    bpt_bt = -(-cnt.max(axis=0) // P)  # [NBUCK, tiles]; may be 0 for a bucket
    flat_bpt = bpt_bt.reshape(-1)  # stream order (bucket-major)
    colstart = np.zeros(NBUCK * tiles + 1, dtype=np.int64)
    np.cumsum(flat_bpt, out=colstart[1:])
    nb = int(colstart[-1])

    order = np.argsort(gkey, kind="stable")
    gs = np.zeros(ngrp + 1, dtype=np.int64)
    np.cumsum(np.bincount(gkey, minlength=ngrp), out=gs[1:])
    pos = np.arange(len(gkey), dtype=np.int64) - gs[gkey[order]]

    ci = core[order]
    bt_flat = (buck * tiles + t_idx)[order]  # stream group id within core
    slot = pos % P
    column = colstart[bt_flat] + pos // P

    val1 = (src - buck * g.bsz1)[order].astype(np.int16)
    val2 = (src2 - buck * g.bsz2)[order].astype(np.int16)
    assert (src - buck * g.bsz1).max() < 32768 and (src2 - buck * g.bsz2).max() < 32768

    i1 = np.zeros((c, P, nb), dtype=np.int16)
    i2 = np.zeros((c, P, nb), dtype=np.int16)
    dlm = np.zeros((c, P, nb), dtype=np.float32)
    vm = np.zeros((c, P, nb), dtype=np.float32)
    i1[ci, slot, column] = val1
    i2[ci, slot, column] = val2
    dlm[ci, slot, column] = dl[order]
    vm[ci, slot, column] = norm[order]

    # gather calls: chunk each bucket's column range into <=gcols-block calls
    calls = []
    for b in range(NBUCK):
        cs, ce = int(colstart[b * tiles]), int(colstart[(b + 1) * tiles])
        c0 = cs
        while c0 < ce:
            k = min(g.gcols, ce - c0)
            calls.append((c0, k, b))
            c0 += k

    def wrap16(mat):  # [P, nb] msg-block values -> dma_gather idx layout
        out = np.zeros((P, nb * 8), dtype=np.int16)
        for c0, k, _b in calls:
            seg = mat[:, c0 : c0 + k].T.reshape(-1)  # call msgs j = m - c0*128
            w = seg.reshape(k * 8, 16).T  # [16, k*8]
            out[:16, c0 * 8 : (c0 + k) * 8] = w
        return np.tile(out[:16], (8, 1))  # replicate across 8 q7 cores

    per_core = [
        dict(gidx1=wrap16(i1[i]), gidx2=wrap16(i2[i]), dl=dlm[i], v=vm[i])
        for i in range(c)
    ]
    layout = dict(
        bpt_bt=[[int(x) for x in row] for row in bpt_bt],
        colstart=[int(x) for x in colstart],
        calls=calls,
        nb=nb,
    )
    return per_core, layout


def build_program(g: Geom, layout):
    import concourse.bass as bass  # noqa: F401
    import concourse.mybir as mybir
    import concourse.tile as tile
    from concourse import bacc, library_config

    f32 = mybir.dt.float32
    i16 = mybir.dt.int16
    mm_dt = mybir.dt.bfloat16 if g.mm_bf16 else mybir.dt.float32

    nb = layout["nb"]
    bpt_bt = layout["bpt_bt"]
    colstart = layout["colstart"]
    calls = layout["calls"]
    shard, tiles, shard_pad = g.shard, g.tiles, g.shard_pad
    ablate = set(os.environ.get("GCN_ABLATE", "").split(","))  # timing experiments
    stage = os.environ.get("GCN_STAGE", "full")  # g | gs | gsm | full

    # col -> call id
    col2call = np.zeros(nb, dtype=np.int64)
    for ci_, (c0, k, _b) in enumerate(calls):
        col2call[c0 : c0 + k] = ci_

    nc = bacc.Bacc(
        "TRN2",
        target_bir_lowering=False,
        debug=False,
        enable_asserts=False,
        num_devices=g.n_cores,
        num_swdge_queues=4,
        dynamic_dma_scratch_size=int(os.environ.get("GCN_SCRATCH", "65536")),
    )

    x_d = nc.dram_tensor("x", [g.n_nodes, g.in_dim], mm_dt, kind="ExternalInput")
    gi1_d = nc.dram_tensor("gidx1", [P, nb * 8], i16, kind="ExternalInput")
    gi2_d = nc.dram_tensor("gidx2", [P, nb * 8], i16, kind="ExternalInput")
    dl_d = nc.dram_tensor("dl", [P, nb], f32, kind="ExternalInput")
    v_d = nc.dram_tensor("v", [P, nb], f32, kind="ExternalInput")
    w1_d = nc.dram_tensor("w1", [g.in_dim, g.h1], mm_dt, kind="ExternalInput")
    w2_d = nc.dram_tensor("w2", [g.h1, g.h2], mm_dt, kind="ExternalInput")
    b1_d = nc.dram_tensor("b1", [g.h1], f32, kind="ExternalInput")
    b2_d = nc.dram_tensor("b2", [g.h2], f32, kind="ExternalInput")
    io_d = nc.dram_tensor("iotam", [P, P], f32, kind="ExternalInput")
    idm_d = nc.dram_tensor("identm", [P, P], mm_dt, kind="ExternalInput")
    idf_d = nc.dram_tensor("identf", [P, P], f32, kind="ExternalInput")
    out_d = nc.dram_tensor("out", [shard, g.h2], f32, kind="ExternalOutput")

    hb_d = nc.dram_tensor("h_bounce", [shard_pad, g.h1], mm_dt, kind="Internal")
    _shared = "Local" if os.environ.get("GCN_NOSHARED", "0") == "1" else "Shared"
    hf_d = nc.dram_tensor(
        "h_full",
        [g.n_cores * shard_pad, g.h1],
        mm_dt,
        kind="Internal",
        addr_space=_shared,
    )

    with tile.TileContext(nc) as tc:
        with (
            tc.tile_pool(name="const", bufs=1) as cpool,
            tc.tile_pool(name="msg", bufs=int(os.environ.get("GCN_MBUFS", "8"))) as mpool,
            tc.tile_pool(name="sel", bufs=6) as spool,
            tc.tile_pool(name="act", bufs=3) as apool,
            tc.tile_pool(name="psum", bufs=2, space="PSUM") as ppool,
        ):
            nc.gpsimd.load_library(library_config.mlp)

            iota_f = cpool.tile([P, P], f32, tag="iota_f")
            nc.sync.dma_start(out=iota_f[:], in_=io_d[:, :])
            ident = cpool.tile([P, P], mm_dt, tag="ident")
            nc.sync.dma_start(out=ident[:], in_=idm_d[:, :])
            ident_f = cpool.tile([P, P], f32, tag="ident_f")
            nc.sync.dma_start(out=ident_f[:], in_=idf_d[:, :])

            w1_s = cpool.tile([g.in_dim, g.h1], mm_dt, tag="w1")
            nc.sync.dma_start(out=w1_s[:], in_=w1_d[:, :])
            w2_s = cpool.tile([g.h1, g.h2], mm_dt, tag="w2")
            nc.sync.dma_start(out=w2_s[:], in_=w2_d[:, :])
            b1_s = cpool.tile([g.h1, 1], f32, tag="b1")
            nc.sync.dma_start(out=b1_s[:], in_=b1_d[:, None])
            b2_s = cpool.tile([g.h2, 1], f32, tag="b2")
            nc.sync.dma_start(out=b2_s[:], in_=b2_d[:, None])

            gidx_s = cpool.tile([P, nb * 8], i16, tag="gidx")
            dl_s = cpool.tile([P, nb], f32, tag="dl")
            nc.sync.dma_start(out=dl_s[:], in_=dl_d[:, :])
            v_s = cpool.tile([P, nb], f32, tag="v")
            nc.sync.dma_start(out=v_s[:], in_=v_d[:, :])

            def layer(gi_dram, table_bucket_ap, feat, w_s, outw, bias_s, out_dt, store):
                nc.sync.dma_start(out=gidx_s[:], in_=gi_dram[:, :])
                msg_tiles: dict[int, object] = {}

                def ensure_call(ci_: int):
                    if ci_ in msg_tiles:
                        return
                    c0, k, b = calls[ci_]
                    mt = mpool.tile([P, g.gcols * feat], mm_dt, tag="msg")
                    if "gather" in ablate:
                        msg_tiles[ci_] = mt
                        return
                    nc.gpsimd.dma_gather(
                        queue_num=ci_ % 4,
                        out_ap=mt[:].rearrange("p (k d) -> p k d", d=feat)[:, :k, :],
                        in_ap=table_bucket_ap(b),
                        idxs_ap=gidx_s[:, c0 * 8 : (c0 + k) * 8],
                        num_idxs=k * P,
                        num_idxs_reg=k * P,
                        elem_size=feat,
                        # single_packet=True wedges the exec unit above
                        # ~1024 indices per call (HW-probed)
                        single_packet=False,
                    )
                    msg_tiles[ci_] = mt

                for t in range(tiles):
                    blocks = [
                        (b, colstart[b * tiles + t] + blk)
                        for b in range(NBUCK)
                        for blk in range(bpt_bt[b][t])
                    ]
                    assert blocks, f"tile {t} has no message blocks"
                    if stage in ("gsm", "full"):
                        p1 = ppool.tile([P, P], f32, tag="p1", space="PSUM")
                    for i_, (b, col) in enumerate(blocks):
                        ci_ = int(col2call[col])
                        ensure_call(ci_)
                        if ci_ + 1 < len(calls) and col - calls[ci_][0] >= calls[ci_][1] - 3:
                            ensure_call(ci_ + 1)
                        if stage == "g":
                            continue
                        off = col - calls[ci_][0]
                        s_t = spool.tile([P, P], mm_dt, tag="S")
                        nc.vector.tensor_scalar(
                            s_t[:],
                            iota_f[:],
                            dl_s[:, col : col + 1],
                            v_s[:, col : col + 1],
                            op0=mybir.AluOpType.is_equal,
                            op1=mybir.AluOpType.mult,
                        )
                        if stage == "gs":
                            continue
                        nc.tensor.matmul(
                            p1[:],
                            lhsT=msg_tiles[ci_][:, off * feat : (off + 1) * feat],
                            rhs=s_t[:],
                            start=(i_ == 0),
                            stop=(i_ == len(blocks) - 1),
                        )
                    if stage in ("g", "gs", "gsm"):
                        continue
                    a1 = apool.tile([P, P], mm_dt, tag="a1")
                    nc.vector.tensor_copy(a1[:feat, :], p1[:feat, :])
                    p2 = ppool.tile([P, P], f32, tag="p2", space="PSUM")
                    nc.tensor.matmul(
                        p2[:outw, :], lhsT=w_s[:feat, :outw], rhs=a1[:feat, :],
                        start=True, stop=True,
                    )
                    ht = apool.tile([P, P], out_dt, tag="ht")
                    nc.scalar.activation(
                        ht[:outw, :], p2[:outw, :],
                        mybir.ActivationFunctionType.Relu,
                        bias=bias_s[:outw, :],
                    )
                    pt = ppool.tile([P, P], out_dt, tag="pt", space="PSUM")
                    idn = ident if out_dt == mm_dt else ident_f
                    nc.tensor.transpose(
                        pt[:, :outw], ht[:outw, :], idn[:outw, :outw]
                    )
                    hrow = apool.tile([P, P], out_dt, tag="hrow")
                    nc.vector.tensor_copy(hrow[:, :outw], pt[:, :outw])
                    store(t, hrow)

            def store_l1(t, hrow):
                nc.sync.dma_start(out=hb_d[t * P : (t + 1) * P, :], in_=hrow[:, : g.h1])

            def store_l2(t, hrow):
                rows = min(P, shard - t * P)
                nc.sync.dma_start(
                    out=out_d[t * P : t * P + rows, :], in_=hrow[:rows, : g.h2]
                )

            def tab1(b):
                lo = b * g.bsz1
                hi = min(g.n_nodes, lo + g.bsz1)
                return x_d[lo:hi, :]

            def tab2(b):
                lo = b * g.bsz2
                hi = min(g.n_cores * shard_pad, lo + g.bsz2)
                return hf_d[lo:hi, :]

            layer(gi1_d, tab1, g.in_dim, w1_s, g.h1, b1_s, mm_dt, store_l1)

            tc.strict_bb_all_engine_barrier()
            if os.environ.get("GCN_NOCC", "0") == "1":  # debug: skip collective
                nc.sync.dma_start(out=hf_d[:shard_pad, :], in_=hb_d[:, :])
            else:
                # bf16 AllGather was observed to wedge the exec unit at
                # >=512KB per rank; it is pure data movement, so ship the
                # same bytes as f32.
                cc_in = hb_d.ap() if not g.mm_bf16 else hb_d.ap().bitcast(f32)
                cc_out = hf_d.ap() if not g.mm_bf16 else hf_d.ap().bitcast(f32)
                nc.gpsimd.collective_compute(
                    "AllGather",
                    mybir.AluOpType.bypass,
                    replica_groups=[list(range(g.n_cores))],
                    ins=[cc_in.opt()],
                    outs=[cc_out.opt()],
                )
            tc.strict_bb_all_engine_barrier()

            layer(gi2_d, tab2, g.h1, w2_s, g.h2, b2_s, f32, store_l2)

    nc.compile()
    return nc


_PROGRAM_CACHE: dict = {}
LAST_RESULTS = None  # BassKernelResults of the most recent kernel() call


def _layout_key(layout):
    return (
        tuple(tuple(r) for r in layout["bpt_bt"]),
        tuple(layout["calls"]),
    )


def _get_program(g: Geom, layout):
    key = (g, _layout_key(layout))
    if key not in _PROGRAM_CACHE:
        _PROGRAM_CACHE[key] = build_program(g, layout)
    return _PROGRAM_CACHE[key]


def host_consts(g: Geom):
    import ml_dtypes

    tdt = ml_dtypes.bfloat16 if g.mm_bf16 else np.float32
    iotam = np.tile(np.arange(P, dtype=np.float32), (P, 1))
    ident = np.eye(P, dtype=np.float32)
    return dict(iotam=iotam, identm=ident.astype(tdt), identf=ident)


def run(x, edge_index, W1, b1, W2, b2, g: Geom, trace: bool = False):
    global LAST_RESULTS
    import ml_dtypes
    from concourse.bass_utils import run_bass_kernel_spmd

    per_core, layout = preprocess(np.asarray(edge_index), g)
    nc = _get_program(g, layout)

    tdt = ml_dtypes.bfloat16 if g.mm_bf16 else np.float32
    consts = host_consts(g)
    x_t = np.ascontiguousarray(np.asarray(x)).astype(tdt)
    w1_t = np.asarray(W1).astype(tdt)
    w2_t = np.asarray(W2).astype(tdt)
    b1_t = np.asarray(b1).astype(np.float32)
    b2_t = np.asarray(b2).astype(np.float32)

    in_maps = [
        dict(
            x=x_t, gidx1=pc["gidx1"], gidx2=pc["gidx2"], dl=pc["dl"], v=pc["v"],
            w1=w1_t, w2=w2_t, b1=b1_t, b2=b2_t, **consts,
        )
        for pc in per_core
    ]

    core_ids = list(range(g.n_cores))
    if trace:
        try:
            res = run_bass_kernel_spmd(
                nc, in_maps, core_ids=core_ids, trace=True, trace_cores=[0]
            )
        except Exception as e:  # fall back to an untraced run
            print(f"[kernel] traced run failed ({type(e).__name__}: {e}); retrying untraced")
            res = run_bass_kernel_spmd(nc, in_maps, core_ids=core_ids)
    else:
        res = run_bass_kernel_spmd(nc, in_maps, core_ids=core_ids)
    LAST_RESULTS = res
    out = np.concatenate([r["out"] for r in res.results], axis=0)
    return out[: g.n_nodes]


_FULL = Geom(
    n_nodes=100000,
    n_cores=8,
    in_dim=128,
    h1=128,
    h2=64,
    gcols=int(os.environ.get("GCN_GCOLS", "24")),
    mm_bf16=os.environ.get("GCN_F32", "0") != "1",
)


def kernel(x, edge_index, W1, b1, W2, b2):
    trace = os.environ.get("GCN_TRACE", "0") == "1"
    return run(x, edge_index, W1, b1, W2, b2, _FULL, trace=trace)

